# revision 25
# baseline (speedup 1.0000x reference)
"""Trainium2 Bass kernel: per-timestep dense softmax attention (frame + memory).

Problem (hardcoded): B=2, T=8, HW=4096, C=64, Cv=3, M=1024, fp32.
  out[b,t] = 0.8 * softmax(kj @ ki^T) @ vi  +  0.2 * softmax(kj @ mk^T) @ mv
with kj = k[b,t+1] (queries), ki = k[b,t] (keys), vi = v[b,t].

Sharding: 8 cores = 2 batches x 4 query-blocks of 1024 rows. Each core handles
all 7 timesteps for its (b, q-range): perfectly balanced.

On-device layout (per core, per step):
  - keys transposed to [C=64 part, keyidx free] via PE transpose (identity mm)
  - logits^T tiles [128 keys part, 1024 q free] = ki_chunk @ kj^T  (bf16 matmul)
  - exp on ACT straight out of PSUM -> SBUF (no max subtraction: logits are
    bounded ~+-50 so exp fits fp32 comfortably)
  - AV matmul with ones column appended to v: lhsT = v1 [128 keys, 4], rhs =
    exp tile -> accumulates numerator rows 0..2 and denominator row 3 into a
    PSUM accumulator; frame attention in col-group 0 (rows 0..3), memory
    attention in col-group 1 (rows 32..35).
  - normalization + 0.8/0.2 combine + transpose to [q, 3] done on host (tiny).
"""

import numpy as np

import concourse.bacc as bacc
import concourse.bass as bass
import concourse.tile as tile
from concourse import masks, mybir
from concourse.bass_utils import run_bass_kernel_spmd

B, T, HW, C, Cv, M = 2, 8, 4096, 64, 3, 1024
TS = T - 1  # 7 steps
QB = HW // 4  # 1024 queries per core
NKC = HW // 128  # 32 frame key chunks
NMC = M // 128  # 8 memory key chunks
COEF = 0.2

F32 = mybir.dt.float32
F32R = mybir.dt.float32r
BF16 = mybir.dt.bfloat16
AF = mybir.ActivationFunctionType

_CACHE = {}
TRACE = False


def _build_nc(repeat=1, mode="full"):
    nc = bacc.Bacc("TRN2", target_bir_lowering=False)

    kf = nc.dram_tensor("kf", [TS, HW, C], F32, kind="ExternalInput")
    kq = nc.dram_tensor("kq", [TS, QB, C], F32, kind="ExternalInput")
    mk = nc.dram_tensor("mk", [TS, M, C], F32, kind="ExternalInput")
    vf = nc.dram_tensor("vf", [TS, HW, Cv], F32, kind="ExternalInput")
    mv = nc.dram_tensor("mv", [TS, M, Cv], F32, kind="ExternalInput")
    out = nc.dram_tensor("out", [TS, 16, QB], F32, kind="ExternalOutput")

    with tile.TileContext(nc) as tc:
        with (
            tc.tile_pool(name="singles", bufs=1) as singles,
            tc.tile_pool(name="stage_kf", bufs=3) as stage_kf_p,
            tc.tile_pool(name="stage_sm", bufs=2) as stage_sm_p,
            tc.tile_pool(name="kiT", bufs=3) as kiT_p,
            tc.tile_pool(name="kjT", bufs=2) as kjT_p,
            tc.tile_pool(name="mkT", bufs=2) as mkT_p,
            tc.tile_pool(name="expp", bufs=44) as exp_p,
            tc.tile_pool(name="ostage", bufs=2) as ost_p,
            tc.tile_pool(name="oddstage", bufs=2) as odd_p,
            tc.tile_pool(name="ps_l", bufs=2, space="PSUM") as ps_l_p,
            tc.tile_pool(name="ps_acc", bufs=1, space="PSUM") as ps_acc_p,
            tc.tile_pool(name="ps_t", bufs=2, space="PSUM") as ps_t_p,
        ):
            identity = singles.tile([128, 128], F32)
            masks.make_identity(nc, identity[:])

            # v with ones column appended: v1[p, t, c, 0:3] = v[t, c*128+p, :],
            # v1[p, t, c, 3] = 1.0 ; same for memory values. Staged through an
            # f32 tile then DVE-copied so the bf16 rounding happens on-engine.
            v1 = singles.tile([128, TS, NKC, 4], BF16)
            mv1 = singles.tile([128, TS, NMC, 4], BF16)
            v1s = singles.tile([128, TS, NKC, 4], F32)
            mv1s = singles.tile([128, TS, NMC, 4], F32)
            nc.vector.memset(v1s[:, :, :, 3:4], 1.0)
            nc.vector.memset(mv1s[:, :, :, 3:4], 1.0)
            for t in range(TS):
                nc.gpsimd.dma_start(
                    out=v1s[:, t, :, 0:3],
                    in_=vf[t].rearrange("(c p) d -> p c d", p=128),
                )
                nc.gpsimd.dma_start(
                    out=mv1s[:, t, :, 0:3],
                    in_=mv[t].rearrange("(c p) d -> p c d", p=128),
                )
                nc.vector.tensor_copy(v1[:, t], v1s[:, t])
                nc.vector.tensor_copy(mv1[:, t], mv1s[:, t])

            if mode != "full":
                ost0 = ost_p.tile([40, QB], F32, tag="ost")
                nc.vector.memset(ost0[:], 0.0)
                for t in range(TS):
                    nc.sync.dma_start(out=out[t, 0:4], in_=ost0[0:4, :])
                    nc.sync.dma_start(out=out[t, 4:8], in_=ost0[32:36, :])
            for _rep in range(repeat):
              for t in range(TS):
                # ---- load this step's keys/queries/memories ----
                skf = stage_kf_p.tile([128, NKC, C], F32)
                nc.sync.dma_start(
                    out=skf, in_=kf[t].rearrange("(c p) x -> p c x", p=128)
                )
                skq = stage_sm_p.tile([128, QB // 128, C], F32, tag="skq")
                nc.sync.dma_start(
                    out=skq, in_=kq[t].rearrange("(c p) x -> p c x", p=128)
                )
                smk = stage_sm_p.tile([128, NMC, C], F32, tag="smk")
                nc.sync.dma_start(
                    out=smk, in_=mk[t].rearrange("(c p) x -> p c x", p=128)
                )
                if mode == "dma":
                    continue

                # ---- transpose to [C, keys] layout, chunk-pairs packed into
                # partition halves: kiT[:, j*128:(j+1)*128] holds chunk 2j in
                # partitions 0:64 and chunk 2j+1 in partitions 64:128 (bottom
                # halves assembled via SBUF->SBUF DMA; PE transpose-mode can
                # only write partitions 0:64). kjT bottom duplicates the top.
                kiT = kiT_p.tile([128, HW // 2], BF16)
                kjT = kjT_p.tile([128, QB], BF16)
                mkT = mkT_p.tile([128, M // 2], BF16)
                odd_kf = odd_p.tile([64, HW // 2], BF16, tag="okf")
                odd_mk = odd_p.tile([64, M // 2], BF16, tag="omk")
                for half, dst in ((0, kiT), (1, odd_kf)):
                    for g in range(4):  # 4 groups x 4 chunks per half
                        pst = ps_t_p.tile([64, 512], F32, tag="pst")
                        for j4 in range(4):
                            c = 2 * (g * 4 + j4) + half
                            nc.tensor.transpose(
                                pst[:, j4 * 128 : (j4 + 1) * 128],
                                skf[:, c, :], identity[:],
                            )
                        nc.vector.tensor_copy(
                            dst[0:64, g * 512 : (g + 1) * 512], pst[:]
                        )
                nc.sync.dma_start(out=kiT[64:128, :], in_=odd_kf[:])
                for g in range(2):
                    pst = ps_t_p.tile([64, 512], F32, tag="pst")
                    for j4 in range(4):
                        c = g * 4 + j4
                        nc.tensor.transpose(
                            pst[:, j4 * 128 : (j4 + 1) * 128],
                            skq[:, c, :], identity[:],
                        )
                    nc.vector.tensor_copy(
                        kjT[0:64, g * 512 : (g + 1) * 512], pst[:]
                    )
                nc.sync.dma_start(out=kjT[64:128, :], in_=kjT[0:64, :])
                for half, dst in ((0, mkT), (1, odd_mk)):
                    pst = ps_t_p.tile([64, 512], F32, tag="pst")
                    for j4 in range(4):
                        c = 2 * j4 + half
                        nc.tensor.transpose(
                            pst[:, j4 * 128 : (j4 + 1) * 128],
                            smk[:, c, :], identity[:],
                        )
                    nc.vector.tensor_copy(dst[0:64, :], pst[:])
                nc.sync.dma_start(out=mkT[64:128, :], in_=odd_mk[:])
                if mode == "trans":
                    continue

                # ---- phase 1: logits + exp for all 40 chunks (20 pairs) ----
                # row-packed: top half (rows 0:64) computes chunk 2j, bottom
                # half (rows 64:128, tile_position=(64,0)) computes chunk 2j+1,
                # so each LDWEIGHTS overlaps the other half's matmuls.
                NPAIR = (NKC + NMC) // 2
                extiles = []
                for j in range(NPAIR):
                    srcT = kiT if j < 16 else mkT
                    col = j * 128 if j < 16 else (j - 16) * 128
                    exa = exp_p.tile([128, QB], BF16, tag="ex")
                    exb = exp_p.tile([128, QB], BF16, tag="ex")
                    psa = ps_l_p.tile([128, QB], F32, tag="psl")
                    for h in range(2):
                        sl = slice(h * 512, (h + 1) * 512)
                        nc.tensor.matmul(
                            psa[:, sl],
                            lhsT=srcT[0:64, col : col + 128],
                            rhs=kjT[0:64, sl],
                            start=True, stop=True,
                        )
                    psb = ps_l_p.tile([128, QB], F32, tag="psl")
                    for h in range(2):
                        sl = slice(h * 512, (h + 1) * 512)
                        nc.tensor.matmul(
                            psb[:, sl],
                            lhsT=srcT[64:128, col : col + 128],
                            rhs=kjT[64:128, sl],
                            start=True, stop=True,
                            tile_position=(64, 0),
                        )
                    if mode != "mm2":
                        nc.scalar.activation(exa[:], psa[:], AF.Exp)
                        nc.scalar.activation(exb[:], psb[:], AF.Exp)
                    extiles.append((exa, exb))
                if mode in ("act", "mm2"):
                    continue

                # ---- phase 2: AV matmuls, alternating PSUM col groups so
                # each tiny v-weight load overlaps the other group's matmul.
                # acc rows 0:4 = frame-even, 32:36 = frame-odd,
                #          64:68 = memory-even, 96:100 = memory-odd.
                acc = ps_acc_p.tile([128, QB], F32)
                for j in range(NPAIR):
                    exa, exb = extiles[j]
                    for half, ex in ((0, exa), (1, exb)):
                        kc = 2 * j + half
                        if j < 16:
                            lhs_v = v1[:, t, kc, :]
                            row = 32 * half
                            start = j == 0
                            stop = j == 15
                        else:
                            lhs_v = mv1[:, t, kc - NKC, :]
                            row = 64 + 32 * half
                            start = j == 16
                            stop = j == NPAIR - 1
                        for h in range(2):
                            sl = slice(h * 512, (h + 1) * 512)
                            nc.tensor.matmul(
                                acc[row : row + 4, sl],
                                lhsT=lhs_v,
                                rhs=ex[:, sl],
                                start=start,
                                stop=stop,
                                tile_position=(0, row),
                                skip_group_check=True,
                            )

                ost = ost_p.tile([128, QB], F32, tag="ost")
                for row in (0, 32, 64, 96):
                    nc.vector.tensor_copy(
                        ost[row : row + 4, :], acc[row : row + 4, :]
                    )
                for i, row in enumerate((0, 32, 64, 96)):
                    nc.sync.dma_start(
                        out=out[t, 4 * i : 4 * i + 4], in_=ost[row : row + 4, :]
                    )
    nc.finalize()
    return nc


def _make_sharded(nc, n_cores=8):
    """Build the shard_map'd jitted callable once, mirroring
    bass2jax.run_bass_via_pjrt, so repeated timed executions reuse the
    compiled executable and device-resident inputs."""
    import jax
    import jax.numpy as jnp
    from jax.sharding import Mesh, PartitionSpec
    from jax.experimental.shard_map import shard_map
    from concourse import bass2jax, mybir as _mybir

    bass2jax.install_neuronx_cc_hook()
    partition_name = (
        nc.partition_id_tensor.name if nc.partition_id_tensor else None
    )
    in_names, out_names, out_avals, zero_outs = [], [], [], []
    for alloc in nc.m.functions[0].allocations:
        if not isinstance(alloc, mybir.MemoryLocationSet):
            continue
        name = alloc.memorylocations[0].name
        if alloc.kind == "ExternalInput":
            if name != partition_name:
                in_names.append(name)
        elif alloc.kind == "ExternalOutput":
            out_names.append(name)
            shape = tuple(alloc.tensor_shape)
            dtype = _mybir.dt.np(alloc.dtype)
            out_avals.append(jax.core.ShapedArray(shape, dtype))
            zero_outs.append(np.zeros(shape, dtype))
    n_params = len(in_names)
    all_in_names = in_names + out_names
    if partition_name is not None:
        all_in_names.append(partition_name)
    donate = tuple(range(n_params, n_params + len(out_avals)))

    def _body(*args):
        operands = list(args)
        if partition_name is not None:
            operands.append(bass2jax.partition_id_tensor())
        outs = bass2jax._bass_exec_p.bind(
            *operands,
            out_avals=tuple(out_avals),
            in_names=tuple(all_in_names),
            out_names=tuple(out_names),
            lowering_input_output_aliases=(),
            sim_require_finite=True,
            sim_require_nnan=True,
            nc=nc,
        )
        return tuple(outs)

    devices = jax.devices()[:n_cores]
    mesh = Mesh(np.asarray(devices), ("core",))
    sharded = jax.jit(
        shard_map(
            _body, mesh=mesh,
            in_specs=(PartitionSpec("core"),) * (n_params + len(out_avals)),
            out_specs=(PartitionSpec("core"),) * len(out_names),
            check_rep=False,
        ),
        donate_argnums=donate,
        keep_unused=True,
    )
    return sharded, in_names, out_names, zero_outs


def _build_noop_nc():
    nc = bacc.Bacc("TRN2", target_bir_lowering=False)
    x = nc.dram_tensor("x", [128, 128], F32, kind="ExternalInput")
    y = nc.dram_tensor("y", [128, 128], F32, kind="ExternalOutput")
    with tile.TileContext(nc) as tc:
        with tc.tile_pool(name="p", bufs=1) as p:
            t = p.tile([128, 128], F32, tag="t")
            nc.sync.dma_start(out=t[:], in_=x[:])
            nc.sync.dma_start(out=y[:], in_=t[:])
    nc.finalize()
    return nc


def bench_noop(iters=30):
    import time as _time
    import jax

    if "noop_nc" not in _CACHE:
        _CACHE["noop_nc"] = _build_noop_nc()
    nc = _CACHE["noop_nc"]
    sharded, in_names, out_names, zero_outs = _make_sharded(nc)
    x = np.zeros((8 * 128, 128), np.float32)
    dev_in = [jax.device_put(x)]
    times = []
    for i in range(iters + 3):
        zeros = [np.zeros((8 * z.shape[0], *z.shape[1:]), z.dtype) for z in zero_outs]
        dz = jax.block_until_ready([jax.device_put(z) for z in zeros])
        t0 = _time.perf_counter()
        jax.block_until_ready(sharded(*dev_in, *dz))
        t1 = _time.perf_counter()
        if i >= 3:
            times.append(t1 - t0)
    return times


def bench(k, v, m_k, m_v, iters=30, repeat=1, mode="full"):
    """Time repeated on-device executions; returns per-iter seconds list."""
    import time as _time
    import jax

    k = np.ascontiguousarray(k, dtype=np.float32)
    v = np.ascontiguousarray(v, dtype=np.float32)
    m_k = np.ascontiguousarray(m_k, dtype=np.float32)
    m_v = np.ascontiguousarray(m_v, dtype=np.float32)
    key = f"nc{repeat}_{mode}"
    if key not in _CACHE:
        _CACHE[key] = _build_nc(repeat=repeat, mode=mode)
    nc = _CACHE[key]
    in_maps = []
    for core in range(8):
        b, qc = core // 4, core % 4
        qsl = slice(qc * QB, (qc + 1) * QB)
        in_maps.append({
            "kf": np.ascontiguousarray(k[b, :-1]),
            "kq": np.ascontiguousarray(k[b, 1:, qsl, :]),
            "mk": m_k[b], "vf": np.ascontiguousarray(v[b, :-1]), "mv": m_v[b],
        })
    sharded, in_names, out_names, zero_outs = _make_sharded(nc)
    concat_in = [
        np.concatenate([np.asarray(in_maps[c][n]) for c in range(8)], axis=0)
        for n in in_names
    ]
    dev_in = [jax.device_put(a) for a in concat_in]  # resident once
    times = []
    out = None
    for i in range(iters + 3):
        zeros = [np.zeros((8 * z.shape[0], *z.shape[1:]), z.dtype) for z in zero_outs]
        dz = jax.block_until_ready([jax.device_put(z) for z in zeros])
        t0 = _time.perf_counter()
        out = jax.block_until_ready(sharded(*dev_in, *dz))
        t1 = _time.perf_counter()
        if i >= 3:
            times.append(t1 - t0)
    return times, out


def kernel(k, v, m_k, m_v):
    k = np.ascontiguousarray(k, dtype=np.float32)
    v = np.ascontiguousarray(v, dtype=np.float32)
    m_k = np.ascontiguousarray(m_k, dtype=np.float32)
    m_v = np.ascontiguousarray(m_v, dtype=np.float32)

    if "nc" not in _CACHE:
        _CACHE["nc"] = _build_nc()
    nc = _CACHE["nc"]

    in_maps = []
    for core in range(8):
        b, qc = core // 4, core % 4
        qsl = slice(qc * QB, (qc + 1) * QB)
        in_maps.append(
            {
                "kf": np.ascontiguousarray(k[b, :-1]),
                "kq": np.ascontiguousarray(k[b, 1:, qsl, :]),
                "mk": m_k[b],
                "vf": np.ascontiguousarray(v[b, :-1]),
                "mv": m_v[b],
            }
        )

    res = run_bass_kernel_spmd(nc, in_maps, core_ids=list(range(8)))
    _CACHE["last_result"] = res

    outp = np.empty((B, TS, HW, Cv), dtype=np.float32)
    for core in range(8):
        b, qc = core // 4, core % 4
        o = res.results[core]["out"]  # [TS, 16, QB]
        fk = o[:, 0:4] + o[:, 4:8]    # frame num/den [TS,4,QB]
        mm = o[:, 8:12] + o[:, 12:16]
        nk, dk = fk[:, 0:3], fk[:, 3]
        nm, dm = mm[:, 0:3], mm[:, 3]
        rec = (1.0 - COEF) * nk / dk[:, None, :] + COEF * nm / dm[:, None, :]
        outp[b, :, qc * QB : (qc + 1) * QB, :] = rec.transpose(0, 2, 1)
    return outp
